# revision 2
# baseline (speedup 1.0000x reference)
"""Bass/Trainium2 kernel for BiasedAttention (B=8, N=2048, H=256), SPMD over 8 cores.

Per-core work (data-parallel over batch): one batch element.
  Q = x@Wq*s + bq*s ; K = x@Wk + bk ; V = x@(Wv Wo)   (b_V Wo + b_O folded into bo)
  S^T = K Q^T + attn_bias^T ; P^T = exp(S^T) (unnormalized)
  O = (P @ [V|1]) -> numerator and denominator in one matmul ; out = O/den + bo

Key restructure vs the earlier version: the host stages attn_bias TRANSPOSED
(fp16) and x transposed (fp16), and the score matmul is emitted as
S^T = matmul(lhsT=K^T, rhs=Q^T) so the PSUM output lands with k on
partitions — exactly the layout the PV matmul needs as lhsT. This removes
all 256 PE identity-transpose matmuls (32k cycles) and the f32->bf16
weight/x converts. fp16 staging also halves HBM traffic (the memory
roofline): ~21 MB -> ~11 MB per core.

Q^T/K^T are kept in fp8e4m3 (scale split sqrt(s) on each of Wq/Wk so both
operands sit well inside fp8's normal range) so the score matmuls run in
DoubleRow perf mode, contracting all 256 h in one matmul. P/V are fp16.
"""

import contextlib
import sys

for _p in ("/opt/trn_rl_repo", "/root/.axon_site/_ro/trn_rl_repo"):
    if _p not in sys.path:
        sys.path.append(_p)

import numpy as np

import concourse.bass as bass
import concourse.tile as tile
from concourse import mybir
from concourse.bass_utils import run_bass_kernel_spmd
from concourse.vector_clock import ScopedClock

B, N, H = 8, 2048, 256
SCALE = H ** -0.5
RSCALE = 0.25  # sqrt(SCALE): applied to both Wq and Wk for fp8 range balance
P = 128
NT = N // P          # 16 k tiles
HC = H // P          # 2 h chunks
KC = N // 512        # 4 q chunks (outer stages)
QT = 512 // P        # 4 q tiles per chunk
F32 = mybir.dt.float32
FP16 = mybir.dt.float16
BF16 = mybir.dt.bfloat16
FP8 = mybir.dt.float8e4
DR = mybir.MatmulPerfMode.DoubleRow


def _patch_tile_drain():
    """walrus here rejects >1 sync-wait on a CTRL/Drain instruction; split the
    TileContext exit-drain's waits across a chain of drains."""
    if getattr(tile.TileContext, "_drain_patched", False):
        return

    def _drain_and_barrier(self, tick_clock, wait_clock):
        drain_inst = self.nc.sync.drain()
        wait_clock.add_sem_waits(
            drain_inst.ins, ScopedClock({None: tick_clock.global_clock})
        )
        si = drain_inst.ins.sync_info
        waits = list(si.on_wait) if si is not None and si.on_wait else []
        if len(waits) > 1:
            drain_inst.ins.sync_info = mybir.SyncInfo(on_wait=waits[:1], on_update=[])
            engs = [self.nc.sync, self.nc.vector, self.nc.scalar,
                    self.nc.tensor, self.nc.gpsimd]
            for i, w in enumerate(waits[1:]):
                d2 = engs[i % len(engs)].drain()
                d2.ins.sync_info = mybir.SyncInfo(on_wait=[w], on_update=[])
        self.nc.all_engine_barrier()
        assert self.sems is not None
        popped = self.nc._tile_sem_poison_stack.pop()
        assert popped is self._sem_poison
        self.nc.clear_and_free_semaphores(list(self.sems.allocated().values()))
        self.nc.all_engine_barrier()

    tile.TileContext._drain_and_barrier = _drain_and_barrier
    tile.TileContext._drain_patched = True


MAX_SYNC_WAITS = 1


def _split_sync_waits(nc: bass.Bass, max_waits: int = MAX_SYNC_WAITS):
    """walrus rejects instructions with too many sync waits; hoist the excess
    onto same-engine NOPs inserted just before the instruction."""
    for fn in nc.m.functions:
        for bb in fn.blocks:
            new = []
            for inst in bb.instructions:
                si = inst.sync_info
                waits = list(si.on_wait) if si is not None and si.on_wait else []
                if len(waits) > max_waits:
                    inst.sync_info = mybir.SyncInfo(
                        on_wait=waits[-max_waits:],
                        on_update=list(si.on_update) if si.on_update else [],
                    )
                    excess = waits[:-max_waits]
                    for i in range(0, len(excess), max_waits):
                        nop = mybir.InstNoOp(
                            name=nc.get_next_instruction_name(),
                            sync_info=mybir.SyncInfo(
                                on_wait=excess[i:i + max_waits], on_update=[]
                            ),
                            engine=inst.engine,
                            bass_nofuse=True,
                        )
                        new.append(nop)
                new.append(inst)
            bb.instructions[:] = new


def build_program(repeat: int = 1) -> bass.Bass:
    _patch_tile_drain()
    nc = bass.Bass()
    Exp = mybir.ActivationFunctionType.Exp
    Ident = mybir.ActivationFunctionType.Identity

    # All activations/bias staged fp16 on host (halves the HBM traffic that
    # dominates this memory-regime problem); small per-feature biases stay f32.
    xt_d = nc.declare_dram_parameter("xt", [HC * P, N], FP16, isOutput=False)
    abt_d = nc.declare_dram_parameter("abt", [KC * NT * P, 512], FP16,
                                      isOutput=False)
    w_d = {
        k: nc.declare_dram_parameter(k, [H, H], FP16, isOutput=False)
        for k in ("wq", "wk", "wv")
    }
    b_d = {
        k: nc.declare_dram_parameter(k, [1, H], F32, isOutput=False)
        for k in ("bq", "bk", "bo")
    }
    # fp16 output (half the writeback traffic); host upcasts to f32.
    y_d = nc.declare_dram_parameter("y", [N, H], FP16, isOutput=True)

    with tile.TileContext(nc) as tc:
        with (
            tc.tile_pool(name="const", bufs=1) as const,
            tc.tile_pool(name="rowstage", bufs=1) as rowstage,
            tc.tile_pool(name="acts", bufs=1) as acts,
            tc.tile_pool(name="abt", bufs=3) as abtp,
            tc.tile_pool(name="s", bufs=4) as sp,
            tc.tile_pool(name="ptsb", bufs=2) as ptsb,
            tc.tile_pool(name="small", bufs=4) as small,
            tc.tile_pool(name="ysb", bufs=2) as ysb,
            tc.tile_pool(name="qk", bufs=4, space="PSUM") as qkp,
            tc.tile_pool(name="o", bufs=2, space="PSUM") as op_,
        ):
            loop_cm = (
                tc.For_i(0, repeat, 1) if repeat > 1 else contextlib.nullcontext()
            )
            with loop_cm:
                # ---- SP DMA queue, in consumption order: xt (gates all
                # matmuls), weights/biases, then the bias^T stages ----
                xt_sb = acts.tile([P, HC, N], FP16, name="xt")
                xt_re = xt_d.rearrange("(c p) n -> p c n", p=P)
                nc.sync.dma_start(out=xt_sb[:, 0:1, :], in_=xt_re[:, 0:1, :])
                nc.sync.dma_start(out=xt_sb[:, 1:2, :], in_=xt_re[:, 1:2, :])
                wsb = {}
                for k in ("wq", "wk", "wv"):
                    wt = const.tile([P, HC, H], FP16, name=f"w_{k}")
                    nc.sync.dma_start(
                        out=wt[:], in_=w_d[k].rearrange("(c p) o -> p c o", p=P)
                    )
                    wsb[k] = wt
                bcol = {}
                for k in ("bq", "bk"):
                    bc_ = const.tile([P, HC], F32, name=f"{k}_col")
                    nc.sync.dma_start(
                        out=bc_[:], in_=b_d[k].rearrange("a (c p) -> p (a c)", p=P)
                    )
                    bcol[k] = bc_
                bo_row = rowstage.tile([1, H], F32, name="bo_row")
                nc.sync.dma_start(out=bo_row[:], in_=b_d["bo"][:])
                # bias^T stages, eager in consumption order (bufs=3 makes the
                # j=3 DMA wait on stage-0 consumption, which is long done)
                abt_re = abt_d.rearrange("(j t p) q -> j p t q", j=KC, p=P)
                abt_tiles = [None] * KC
                for j in range(KC):
                    abt_tiles[j] = abtp.tile([P, NT, 512], FP16, name="abt")
                    nc.sync.dma_start(out=abt_tiles[j][:], in_=abt_re[j])

                # ---- Q^T, K^T (fp8e4m3, [h_out part, hc, n]) for DoubleRow ----
                qt = acts.tile([P, HC, N], FP8, name="qt")
                kt = acts.tile([P, HC, N], FP8, name="kt")

                def emit_qtkt_ng(ng):
                    for name_, dst, wkey, bkey in (
                        ("qt", qt, "wq", "bq"), ("kt", kt, "wk", "bk"),
                    ):
                        for ho in range(HC):
                            ps = qkp.tile([P, 512], F32, name="qk")
                            for hi in range(HC):
                                nc.tensor.matmul(
                                    ps[:],
                                    lhsT=wsb[wkey][:, hi, ho * P:(ho + 1) * P],
                                    rhs=xt_sb[:, hi, ng * 512:(ng + 1) * 512],
                                    start=(hi == 0), stop=(hi == HC - 1),
                                )
                            if name_ == "qt":
                                nc.scalar.activation(
                                    dst[:, ho, ng * 512:(ng + 1) * 512], ps[:],
                                    Ident, bias=bcol[bkey][:, ho:ho + 1],
                                )
                            else:
                                nc.vector.tensor_scalar_add(
                                    dst[:, ho, ng * 512:(ng + 1) * 512], ps[:],
                                    bcol[bkey][:, ho:ho + 1],
                                )

                # ---- V_ext (fp16, [k part, t, h | ones]); b_V folded out ----
                v_sb = acts.tile([P, NT, H + 1], FP16, name="v")
                nc.vector.memset(v_sb[:, :, H:H + 1], 1.0)

                def emit_v_chunk(t):
                    ps = qkp.tile([P, 512], F32, name="qk")
                    for hi in range(HC):
                        nc.tensor.matmul(
                            ps[:, :H],
                            lhsT=xt_sb[:, hi, t * P:(t + 1) * P],
                            rhs=wsb["wv"][:, hi, :],
                            start=(hi == 0), stop=(hi == HC - 1),
                        )
                    if t % 2 == 0:
                        nc.scalar.copy(v_sb[:, t, :H], ps[:, :H])
                    else:
                        nc.vector.tensor_copy(v_sb[:, t, :H], ps[:, :H])

                # b_O broadcast across partitions via K=1 ones-matmul
                ones_f = const.tile([1, P], F32, name="ones_f")
                nc.vector.memset(ones_f[:], 1.0)
                bo_bc = const.tile([P, H], F32, name="bo_bc")

                for ng in range(KC):
                    emit_qtkt_ng(ng)
                ps_bo = qkp.tile([P, 512], F32, name="qk")
                nc.tensor.matmul(ps_bo[:, :H], lhsT=ones_f[:], rhs=bo_row[:],
                                 start=True, stop=True)
                nc.vector.tensor_copy(bo_bc[:], ps_bo[:, :H])

                # ---- main loop over q chunks of 512, software-pipelined:
                # stage s emits scores for chunk s (16 DR matmuls, one per
                # k tile, each landing S^T[k_tile, q_chunk] in PSUM with k on
                # partitions) interleaved with the PV matmuls for chunk s-1.
                # Stage 0 interleaves the V projections instead. ----
                y_re = y_d.rearrange("(j t p) h -> j p t h", j=KC, p=P)
                pt_tiles = [None] * KC

                def emit_score_chunk(j, t):
                    if t == 0:
                        pt_tiles[j] = ptsb.tile([P, NT, 512], FP16, name="pt")
                    ps = qkp.tile([P, 512], F32, name="qk")
                    nc.tensor.matmul(
                        ps[:],
                        lhsT=kt[:, :, t * P:(t + 1) * P],
                        rhs=qt[:, :, j * 512:(j + 1) * 512],
                        start=True, stop=True,
                        perf_mode=DR,
                    )
                    s_t = sp.tile([P, 512], FP16, name="s")
                    nc.vector.tensor_add(s_t[:], ps[:], abt_tiles[j][:, t, :])
                    nc.scalar.activation(pt_tiles[j][:, t, :], s_t[:], Exp)

                def gen_pv(j):
                    # yields once per PV matmul (64 total); finish/normalize
                    # work is emitted inline between yields
                    pt_t = pt_tiles[j]
                    y_grp = ysb.tile([P, QT, H], FP16, name="y")
                    for qq in range(QT):
                        ps_o = op_.tile([P, 512], F32, name="o")
                        for t in range(NT):
                            nc.tensor.matmul(
                                ps_o[:, :H + 1],
                                lhsT=pt_t[:, t, qq * P:(qq + 1) * P],
                                rhs=v_sb[:, t, :],
                                start=(t == 0), stop=(t == NT - 1),
                            )
                            yield
                        rden = small.tile([P, 1], F32, name="rden")
                        nc.vector.reciprocal(rden[:], ps_o[:, H:H + 1])
                        y1 = small.tile([P, H], F32, name="y1")
                        nc.vector.tensor_scalar_mul(y1[:], ps_o[:, :H], rden[:])
                        nc.gpsimd.tensor_add(y_grp[:, qq, :], y1[:], bo_bc[:])
                        if j == KC - 1:
                            # per-tile output DMAs at the end so the final
                            # transfer is as small/early as possible
                            nc.scalar.dma_start(
                                out=y_re[j][:, qq:qq + 1, :],
                                in_=y_grp[:, qq:qq + 1, :],
                            )
                    if j < KC - 1:
                        nc.scalar.dma_start(out=y_re[j], in_=y_grp[:])

                for s in range(KC + 1):
                    pvg = gen_pv(s - 1) if s >= 1 else None
                    if s < KC:
                        for t in range(NT):
                            emit_score_chunk(s, t)
                            if s == 0:
                                emit_v_chunk(t)
                            if pvg is not None:
                                for _ in range(QT):
                                    try:
                                        next(pvg)
                                    except StopIteration:
                                        break
                    if pvg is not None:
                        for _ in pvg:
                            pass

    _split_sync_waits(nc)
    return nc


_NC = None


def _get_program():
    global _NC
    if _NC is None:
        _NC = build_program()
    return _NC


def make_in_maps(x, attn_bias, W_Q, b_Q, W_K, b_K, W_V, b_V, W_O, b_O):
    f = np.float32
    f16 = np.float16
    W_V, W_O = np.asarray(W_V, np.float64), np.asarray(W_O, np.float64)
    b_V = np.asarray(b_V, np.float64)
    shared = {
        # sqrt(SCALE) on each of Wq/Wk: centers both Q and K in fp8 range
        "wq": np.ascontiguousarray((np.asarray(W_Q, f) * RSCALE).astype(f16)),
        "wk": np.ascontiguousarray((np.asarray(W_K, f) * RSCALE).astype(f16)),
        # W_O folded into the V projection: y = (P @ [V Wo | 1])/den + bo
        "wv": np.ascontiguousarray((W_V @ W_O).astype(f16)),
        "bq": np.asarray(b_Q, f).reshape(1, H) * RSCALE,
        "bk": np.asarray(b_K, f).reshape(1, H) * RSCALE,
        # b_V folded through W_O into the output bias
        "bo": ((b_V @ W_O).astype(f) + np.asarray(b_O, f)).reshape(1, H),
    }
    x = np.asarray(x, f)
    ab = np.asarray(attn_bias, f)
    maps = []
    for b in range(B):
        xt = np.ascontiguousarray(x[b].T.astype(f16))  # [H, N]
        abt = ab[b].T.astype(f16)                      # [k, q]
        # stage bias^T as [q_chunk j][k_tile t][k within tile p][q within
        # chunk]: one contiguous 2 MB DMA per outer stage
        abts = np.ascontiguousarray(
            abt.reshape(NT, P, KC, 512).transpose(2, 0, 1, 3)
        ).reshape(KC * NT * P, 512)
        maps.append({"xt": xt, "abt": abts, **shared})
    return maps


def kernel(x, attn_bias, W_Q, b_Q, W_K, b_K, W_V, b_V, W_O, b_O, _trace=False):
    nc = _get_program()
    in_maps = make_in_maps(x, attn_bias, W_Q, b_Q, W_K, b_K, W_V, b_V, W_O, b_O)
    res = run_bass_kernel_spmd(nc, in_maps, core_ids=list(range(B)), trace=_trace)
    out = np.stack(
        [np.asarray(res.results[b]["y"], dtype=np.float32) for b in range(B)],
        axis=0,
    )
    if _trace:
        kernel.last_results = res
    return out
